# revision 1
# baseline (speedup 1.0000x reference)
"""DirectionalContrastiveLoss on 8 TRN2 NeuronCores (Bass/Tile).

Data-parallel over the N=16384 anchor rows (2048 rows/core); the 4000-row
memory bank is replicated (padded to 4096 columns with zero features).

Device algorithm (validated in numcheck.py):
- sim = feat @ memT/TEMP - 1000*eq, computed on the PE in bf16: two K=128
  feature tiles per output chunk, plus the label mask as bf16
  -1000*onehot(label) x onehot(mem_label) matmuls placed on per-unit
  32-row PE tile positions.  exp(sim-1000) == 0 in fp32, which reproduces
  the reference's masked exp-sum exactly.
- The softmax shift is simply pos (not the row max): rows where
  exp(sim - pos) overflows to +inf are provably dead (sim >= pos + 88
  implies the true logit < e^-88, so -log(sigma+EPS) = -log(EPS) either
  way), and rows that matter (pos within ~18 of the max) can never
  overflow.  So sigma = 1/(sum exp(sim-pos) + 1 + EPS) with no row max,
  no rescaling passes.
- Per-unit exp+accumulate runs on ScalarE (unit 0) while VectorE
  reduce-sums the other three units' exp'd PSUM, balancing the engines.
Each core returns [128, 4] partial sums (num1, den1, num2, den2); the
host does the final reduction and division.
"""
from contextlib import ExitStack

import numpy as np
import ml_dtypes

TEMP = 0.1
POS_THRESH = 0.7
EPS = 1e-8
N, C, M, NLAB = 16384, 256, 4000, 21
MP = 4096                  # memory columns padded
NCORES = 8
RPC = N // NCORES          # 2048 rows per core
NT = RPC // 128            # 16 n-tiles per core
NU = 4                     # psum units per n-tile
UNIT = MP // NU            # 1024 (= 2 PSUM banks, J=512 chunks)

_cache = {}


def _build():
    import concourse.bacc as bacc
    import concourse.tile as tile
    from concourse import mybir

    f32 = mybir.dt.float32
    bf16 = mybir.dt.bfloat16
    f8 = mybir.dt.float8e4
    Alu = mybir.AluOpType
    Act = mybir.ActivationFunctionType
    X = mybir.AxisListType.X
    DR = mybir.MatmulPerfMode.DoubleRow

    # Bacc (not raw Bass): its finalize() runs generate_event_semaphores(),
    # which splits multi-sem waits into EVSEM chains — walrus allows at most
    # one sync-wait per instruction.
    nc = bacc.Bacc(None)

    ext1_d = nc.declare_dram_parameter("ext1", [C, RPC], bf16, isOutput=False)
    ext2_d = nc.declare_dram_parameter("ext2", [C, RPC], bf16, isOutput=False)
    eqa1_d = nc.declare_dram_parameter("eqanc1", [128, RPC], bf16, isOutput=False)
    eqa2_d = nc.declare_dram_parameter("eqanc2", [128, RPC], bf16, isOutput=False)
    mem_d = nc.declare_dram_parameter("extmem", [C, MP], bf16, isOutput=False)
    eqm_d = nc.declare_dram_parameter("eqmem", [128, MP], bf16, isOutput=False)
    f1_d = nc.declare_dram_parameter("f1r", [128, NT * C], bf16, isOutput=False)
    f2_d = nc.declare_dram_parameter("f2r", [128, NT * C], bf16, isOutput=False)
    lg1_d = nc.declare_dram_parameter("lg1", [128, NT], f32, isOutput=False)
    lg2_d = nc.declare_dram_parameter("lg2", [128, NT], f32, isOutput=False)
    out_d = nc.declare_dram_parameter("out", [128, 4], f32, isOutput=True)

    with tile.TileContext(nc) as tc, ExitStack() as ctx:
        consts = ctx.enter_context(tc.tile_pool(name="consts", bufs=1))
        small = ctx.enter_context(tc.tile_pool(name="small", bufs=3))
        psum = ctx.enter_context(
            tc.tile_pool(name="psum", bufs=NU, space="PSUM")
        )

        # ---- resident inputs ----
        # Order + chunking matter: tile-0's dependencies are loaded first so
        # the PE starts ~10us in instead of ~27us.  The big memory-bank
        # tensors are split per 1024-column unit so the first matmuls wait
        # only on their own chunk.
        f1t = consts.tile([128, NT, C], bf16, tag="f1t", name="f1t")
        nc.sync.dma_start(out=f1t[:], in_=f1_d[:].rearrange("p (t c) -> p t c", c=C))
        f2t = consts.tile([128, NT, C], bf16, tag="f2t", name="f2t")
        nc.sync.dma_start(out=f2t[:], in_=f2_d[:].rearrange("p (t c) -> p t c", c=C))

        e1_k, e2_k = [], []
        for i in range(2):
            k0, k1 = i * 128, (i + 1) * 128
            t1 = consts.tile([128, RPC], bf16, tag=f"e1_{i}", name=f"e1_{i}")
            nc.sync.dma_start(out=t1[:], in_=ext1_d[k0:k1, :])
            e1_k.append(t1)
        eqa1 = consts.tile([128, RPC], bf16, tag="eqa1", name="eqa1")
        nc.sync.dma_start(out=eqa1[:], in_=eqa1_d[:])

        memc = [[None] * NU for _ in range(2)]
        eqmc = [None] * NU
        for u in range(NU):
            c0, c1 = u * UNIT, (u + 1) * UNIT
            for i in range(2):
                k0, k1 = i * 128, (i + 1) * 128
                mt = consts.tile([128, UNIT], bf16, tag=f"mem{i}u{u}",
                                 name=f"mem{i}u{u}")
                nc.sync.dma_start(out=mt[:], in_=mem_d[k0:k1, c0:c1])
                memc[i][u] = mt
            et = consts.tile([128, UNIT], bf16, tag=f"eqmu{u}", name=f"eqmu{u}")
            nc.sync.dma_start(out=et[:], in_=eqm_d[:, c0:c1])
            eqmc[u] = et
        for i in range(2):
            k0, k1 = i * 128, (i + 1) * 128
            t2 = consts.tile([128, RPC], bf16, tag=f"e2_{i}", name=f"e2_{i}")
            nc.sync.dma_start(out=t2[:], in_=ext2_d[k0:k1, :])
            e2_k.append(t2)
        eqa2 = consts.tile([128, RPC], bf16, tag="eqa2", name="eqa2")
        nc.sync.dma_start(out=eqa2[:], in_=eqa2_d[:])
        lg1t = consts.tile([128, NT], f32, tag="lg1t", name="lg1t")
        nc.sync.dma_start(out=lg1t[:], in_=lg1_d[:])
        lg2t = consts.tile([128, NT], f32, tag="lg2t", name="lg2t")
        nc.sync.dma_start(out=lg2t[:], in_=lg2_d[:])

        outt = consts.tile([128, 4], f32, tag="outt", name="outt")
        epsb = consts.tile([128, 1], f32, tag="epsb", name="epsb")
        nc.vector.memset(epsb[:], EPS)

        # pos (shared by both branches): pos = sum_c (f1/TEMP)*f2
        # (1/TEMP folded into f1r host-side).  NPOS = -pos (the exp bias).
        POS = consts.tile([128, NT], f32, tag="POS", name="POS")
        NPOS = consts.tile([128, NT], f32, tag="NPOS", name="NPOS")
        for t in range(NT):
            scr = small.tile([128, C], f32, tag="posscr", name=f"posscr{t}")
            nc.vector.tensor_mul(scr[:], f1t[:, t, :], f2t[:, t, :])
            nc.vector.reduce_sum(out=POS[:, t : t + 1], in_=scr[:], axis=X)
            nc.vector.tensor_scalar_mul(
                NPOS[:, t : t + 1], POS[:, t : t + 1], -1.0
            )

        for b, (ekt, eqa, lgA, lgB) in enumerate(
            [(e1_k, eqa1, lg1t, lg2t), (e2_k, eqa2, lg2t, lg1t)]
        ):
            SS = consts.tile([128, NT], f32, tag=f"SS{b}", name=f"SS{b}")
            for t in range(NT):
                tc0, tc1 = t * 128, (t + 1) * 128
                pu = [
                    psum.tile([128, UNIT], f32, tag="pu", name=f"pu{b}_{t}_{u}")
                    for u in range(NU)
                ]
                # dense bf16 K=256 feature matmuls (2 K-tiles)
                for kt in range(2):
                    lhsT = ekt[kt][:, tc0:tc1]
                    for u in range(NU):
                        for j in range(2):
                            nc.tensor.matmul(
                                pu[u][:, j * 512 : (j + 1) * 512],
                                lhsT,
                                memc[kt][u][:, j * 512 : (j + 1) * 512],
                                start=(kt == 0),
                                stop=False,
                            )
                # -1000*eq one-hot matmuls (bf16), 4 units on distinct
                # 32-row PE tile positions
                for j in range(2):
                    for u in range(NU):
                        nc.tensor.matmul(
                            pu[u][:, j * 512 : (j + 1) * 512],
                            eqa[32 * u : 32 * u + NLAB, tc0:tc1],
                            eqmc[u][32 * u : 32 * u + NLAB,
                                    j * 512 : (j + 1) * 512],
                            start=False,
                            stop=True,
                            tile_position=(32 * u, 0),
                        )
                # exp(sim - pos) per unit; unit 0 summed by ScalarE accum,
                # units 1..3 by VectorE reduce over the exp'd PSUM
                S = small.tile([128, NU], f32, tag="S", name=f"S{b}_{t}")
                for u in range(NU):
                    nc.scalar.activation(
                        out=pu[u][:],
                        in_=pu[u][:],
                        func=Act.Exp,
                        bias=NPOS[:, t : t + 1],
                        scale=1.0,
                        accum_out=S[:, u : u + 1] if u < 1 else None,
                    )
                for u in range(1, NU):
                    nc.vector.reduce_sum(
                        out=S[:, u : u + 1], in_=pu[u][:], axis=X
                    )
                nc.vector.reduce_sum(out=SS[:, t : t + 1], in_=S[:], axis=X)

            # ---- branch epilogue on [128, NT] ----
            # sigma = 1/(SS + 1 + EPS); loss row = -log(sigma + EPS)
            D = small.tile([128, NT], f32, tag="D", name=f"D{b}")
            nc.vector.tensor_scalar_add(D[:], SS[:], 1.0 + EPS)
            R = small.tile([128, NT], f32, tag="R", name=f"R{b}")
            nc.vector.reciprocal(R[:], D[:])
            LAM = small.tile([128, NT], f32, tag="LAM", name=f"LAM{b}")
            nc.scalar.activation(
                out=LAM[:], in_=R[:], func=Act.Ln, bias=epsb[:], scale=1.0
            )
            A = small.tile([128, NT], f32, tag="A", name=f"A{b}")
            nc.vector.tensor_scalar(
                out=A[:], in0=lgB[:], scalar1=POS_THRESH, scalar2=None,
                op0=Alu.is_gt,
            )
            W = small.tile([128, NT], f32, tag="W", name=f"W{b}")
            nc.vector.tensor_tensor(W[:], lgA[:], lgB[:], op=Alu.is_lt)
            nc.vector.tensor_mul(W[:], W[:], A[:])
            scrN = small.tile([128, NT], f32, tag="scrN", name=f"scrN{b}")
            nc.vector.tensor_mul(scrN[:], LAM[:], W[:])
            nc.vector.reduce_sum(
                out=outt[:, 2 * b : 2 * b + 1], in_=scrN[:], axis=X
            )
            nc.vector.reduce_sum(
                out=outt[:, 2 * b + 1 : 2 * b + 2], in_=W[:], axis=X
            )

        nc.sync.dma_start(out=out_d[:], in_=outt[:])

    nc.finalize()
    return nc


def _host_prep(inputs):
    bf = ml_dtypes.bfloat16
    f8 = ml_dtypes.float8_e4m3
    f1 = np.ascontiguousarray(np.asarray(inputs["output_feat1"], np.float32))
    f2 = np.ascontiguousarray(np.asarray(inputs["output_feat2"], np.float32))
    l1 = np.asarray(inputs["pseudo_label1"], np.int32)
    l2 = np.asarray(inputs["pseudo_label2"], np.int32)
    g1 = np.asarray(inputs["pseudo_logits1"], np.float32)
    g2 = np.asarray(inputs["pseudo_logits2"], np.float32)
    ul1 = np.asarray(inputs["output_ul1"], np.float32)
    ul2 = np.asarray(inputs["output_ul2"], np.float32)
    i1 = np.asarray(inputs["selected_idx1"], np.int64)
    i2 = np.asarray(inputs["selected_idx2"], np.int64)

    b, c, h, w = ul1.shape
    u1 = ul1.transpose(0, 2, 3, 1).reshape(b * h * w, c)
    u2 = ul2.transpose(0, 2, 3, 1).reshape(b * h * w, c)
    mem = np.concatenate([u1[i1], u2[i2]], axis=0)               # [M, C]
    memlab = np.concatenate([l1[i1], l2[i2]], axis=0)            # [M]

    lab_eye = np.arange(NLAB, dtype=np.int32)

    extmem = np.zeros((C, MP), np.float32)
    extmem[:, :M] = mem.T / TEMP
    extmem = extmem.astype(bf)                                   # [256, MP]

    oh_mem = np.zeros((NLAB, MP), np.float32)
    oh_mem[:, :M] = (memlab[None, :] == lab_eye[:, None])
    eqmem = np.zeros((128, MP), np.float32)
    for i in range(NU):
        eqmem[32 * i : 32 * i + NLAB] = oh_mem
    eqmem = eqmem.astype(bf)                                     # [128, MP]

    def eq_anchor(lab):
        oh = -1000.0 * (lab[None, :] == lab_eye[:, None])        # [21, N]
        out = np.zeros((128, lab.shape[0]), np.float32)
        for i in range(NU):
            out[32 * i : 32 * i + NLAB] = oh
        return out.astype(bf)

    ext1 = np.ascontiguousarray(f1.T).astype(bf)                 # [256, N]
    ext2 = np.ascontiguousarray(f2.T).astype(bf)
    eqa1 = eq_anchor(l1)
    eqa2 = eq_anchor(l2)

    def pack_rows(x):   # [RPC, C] -> [128, NT*C]
        return np.ascontiguousarray(
            x.reshape(NT, 128, C).transpose(1, 0, 2).reshape(128, NT * C)
        )

    def pack_vec(v):    # [RPC] -> [128, NT]
        return np.ascontiguousarray(v.reshape(NT, 128).T)

    in_maps = []
    for cix in range(NCORES):
        sl = slice(cix * RPC, (cix + 1) * RPC)
        in_maps.append({
            "ext1": np.ascontiguousarray(ext1[:, sl]),
            "ext2": np.ascontiguousarray(ext2[:, sl]),
            "eqanc1": np.ascontiguousarray(eqa1[:, sl]),
            "eqanc2": np.ascontiguousarray(eqa2[:, sl]),
            "extmem": extmem,
            "eqmem": eqmem,
            "f1r": pack_rows((f1[sl] / TEMP).astype(bf)),
            "f2r": pack_rows(f2[sl].astype(bf)),
            "lg1": pack_vec(g1[sl]),
            "lg2": pack_vec(g2[sl]),
        })
    return in_maps


def _finalize(results):
    num1 = den1 = num2 = den2 = 0.0
    for r in results:
        o = np.asarray(r["out"], np.float64)
        num1 += o[:, 0].sum()
        den1 += o[:, 1].sum()
        num2 += o[:, 2].sum()
        den2 += o[:, 3].sum()
    loss = -(num1 / (den1 + 1e-12) + num2 / (den2 + 1e-12))
    return np.float32(loss)


def _run(inputs, trace=False):
    from concourse.bass_utils import run_bass_kernel_spmd

    if "nc" not in _cache:
        _cache["nc"] = _build()
    in_maps = _host_prep(inputs)
    res = run_bass_kernel_spmd(
        _cache["nc"], in_maps, list(range(NCORES)), trace=trace
    )
    return _finalize(res.results), res


def kernel(**inputs):
    out, _ = _run(inputs)
    return out


def kernel_with_profile(**inputs):
    out, res = _run(inputs, trace=True)
    return out, res

